# revision 59
# baseline (speedup 1.0000x reference)
"""Multi-head causal self-attention (B=2, T=4096, D=768, H=12) on 8 trn2 cores.

Sharding: core c -> batch b = c//4, heads 3*(c%4) .. 3*(c%4)+2.
qkv_proj column-parallel (each core computes Q/K/V only for its heads),
out_proj row-parallel (each core emits a partial y^T; host sums the 4
partials per batch).

Device dataflow (bf16 operands, fp32 PSUM accumulation):
  x^T pre-transposed and cast to bf16 on the host -> Q^T/K^T via
  transposed projection (W^T stationary 128-wide, x^T streaming) ->
  S^T = K Q^T in [k,q] layout, heads 0/1 row-paired on opposite PE
  halves, head 2 self-paired via a partition-swapped copy; diagonal
  band tiles stream only their causally live q columns -> exp straight
  out of PSUM into bf16 SBUF on ScalarE, except head 1 on odd non-band
  k-pairs which uses a DVE integer-bitcast 2^u approx (one tensor_scalar
  into int16 whose bits are the bf16 exp; the +-3% sawtooth cancels in
  the softmax normalization) so neither engine paces the pipeline ->
  causal band masks on DVE (all-SBUF bf16) -> out^T = V^T P^T with a
  ones column appended to V collecting softmax denominators in
  accumulator row 64 -> normalize via approx reciprocal + gpsimd
  partition broadcast -> y^T partial = Wo^T.T out^T with heads 0/1
  row-paired, stored bf16.

Emission order per step: attention hpass0 (h0/h1) -> normalize h0/h1
-> hpass1 (h2, with the next superblock's projection work and this
q-block's first out-proj half interleaved one chunk per k-pair) ->
normalize h2 -> out-proj. The PE queue therefore always has projection
work while the exp/normalize chains run, and AV matmuls lag one k-pair
behind scores so the PE never head-of-line blocks on the exp."""

import sys

sys.path.insert(0, "/opt/trn_rl_repo")

import numpy as np
import ml_dtypes
from contextlib import ExitStack

import concourse.bass as bass
import concourse.bacc as bacc
import concourse.tile as tile
import concourse.mybir as mybir
from concourse.bass_utils import run_bass_kernel_spmd

F32 = mybir.dt.float32
BF16 = mybir.dt.bfloat16
I16 = mybir.dt.int16
AF = mybir.ActivationFunctionType
ALU = mybir.AluOpType
BF = ml_dtypes.bfloat16

B = 2
T = 4096
D = 768
H = 12
DK = 64
NCORES = 8
HL = 3  # heads per core
ND = D // 128  # 6 d-tiles
NKT = T // 128  # 32 k-tiles
NQB = T // 512  # 8 q-blocks
NTSB = T // 512  # 8 t-superblocks (4 t-tiles each)

# DVE bitcast-exp constants: bf16bits(exp(s)) ~ round(s*0.125*log2e*128
# + (16256 - 5.5)); the -5.5 centers the linear-mantissa sawtooth.
EXP_C1 = 0.125 * 1.4426950408889634 * 128.0
EXP_C2 = 16256.0 - 5.5

_CACHE = {}


def _emit(tc):
    nc = tc.nc
    xt_d = nc.dram_tensor("xT", [D, T], BF16, kind="ExternalInput").ap()
    wqk_d = nc.dram_tensor("wqkT", [D, 384], BF16, kind="ExternalInput").ap()
    wv_d = nc.dram_tensor("wvT", [D, HL * DK], BF16, kind="ExternalInput").ap()
    wo_d = nc.dram_tensor("woT", [HL, DK, D], BF16, kind="ExternalInput").ap()
    y_d = nc.dram_tensor("yT", [D, T], BF16, kind="ExternalOutput").ap()
    xt_v = xt_d.rearrange("(j p) t -> p j t", p=128)
    y_v = y_d.rearrange("(j p) t -> p j t", p=128)

    ctx = ExitStack()
    const = ctx.enter_context(tc.tile_pool(name="const", bufs=1))
    persist = ctx.enter_context(tc.tile_pool(name="persist", bufs=1))
    xtpool = ctx.enter_context(tc.tile_pool(name="xt", bufs=3))
    ptpool = ctx.enter_context(tc.tile_pool(name="pt", bufs=6))
    spool = ctx.enter_context(tc.tile_pool(name="sp", bufs=3))
    ypool = ctx.enter_context(tc.tile_pool(name="yp", bufs=3))
    # PSUM (8 banks): pa = streaming (S tiles, qkv, V) 2 bufs x 2 banks;
    # pb = AV accumulators, 2 bufs (h2 reuses h0's slot after normalize
    # h0); pc = yT out-proj, 2 bufs so dj pipelines across the copy.
    psA = ctx.enter_context(tc.tile_pool(name="psA", bufs=2, space="PSUM"))
    psB = ctx.enter_context(tc.tile_pool(name="psB", bufs=2, space="PSUM"))
    psC = ctx.enter_context(tc.tile_pool(name="psC", bufs=2, space="PSUM"))

    # ---- constants ----
    # causal band masks for the 4 diagonal-band k-tiles of each q-block:
    # bandmask[bp][k, q] = 0 for q < 128*bp + k, else 1
    bandmask = []
    for bp in range(4):
        m = const.tile([128, 512], BF16, name=f"bandmask{bp}")
        nc.gpsimd.memset(m, 1.0)
        nc.gpsimd.affine_select(
            out=m, in_=m, compare_op=mybir.AluOpType.is_ge, fill=0.0,
            base=-128 * bp, pattern=[[1, 512]], channel_multiplier=-1,
        )
        bandmask.append(m)

    wqk_sb = const.tile([128, ND, 384], BF16)
    nc.sync.dma_start(out=wqk_sb, in_=wqk_d.rearrange("(j p) e -> p j e", p=128))
    wv_sb = const.tile([128, ND, HL * DK], BF16)
    nc.sync.dma_start(out=wv_sb, in_=wv_d.rearrange("(j p) e -> p j e", p=128))
    wo01_sb = const.tile([128, D], BF16)  # head0 rows on 0:64, head1 on 64:128
    nc.sync.dma_start(out=wo01_sb, in_=wo_d[0:2].rearrange("h p d -> (h p) d"))
    wo2_sb = const.tile([DK, D], BF16)
    nc.sync.dma_start(out=wo2_sb, in_=wo_d[2])

    # ---- persistent activations ----
    # KA: [K^T_h0 ; K^T_h1], QB: [Q^T_h0 ; Q^T_h1] on partition halves
    KA = persist.tile([128, T], BF16, name="KA")
    QB = persist.tile([128, T], BF16, name="QB")
    C2 = persist.tile([128, T], BF16, name="C2")  # [K^T_h2 ; Q^T_h2]
    D2 = persist.tile([128, T], BF16, name="D2")  # [Q^T_h2 ; K^T_h2] (swapped)
    # V natural layout [k-part, kt, head, v(64)+ones]; the ones column
    # collects softmax denominators in AV accumulator row 64
    V = persist.tile([128, NKT, HL, DK + 1], BF16, name="V")
    for h in range(HL):
        nc.gpsimd.memset(V[:, :, h, DK : DK + 1], 1.0)
    ot01 = persist.tile([128, 512], BF16, name="ot01")  # heads 0/1 out^T per qb
    ot2 = persist.tile([DK, 512], BF16, name="ot2")

    qk_dest = [KA, QB, C2]

    def proj_chunks(tsb):
        """Emit the x^T DMA now; return per-(e|t)-tile projection closures
        for interleaved emission."""
        blk = slice(tsb * 512, (tsb + 1) * 512)
        xt_sb = xtpool.tile([128, ND, 512], BF16, name="xt_sb")
        nc.sync.dma_start(out=xt_sb, in_=xt_v[:, :, blk])

        def qk_chunk(et):
            # Q^T / K^T projection: out[e, t] block per e-tile (full 128-wide
            # stationary: e-tile 0 = [K_h0|K_h1], 1 = [Q_h0|Q_h1], 2 = [K_h2|Q_h2])
            def emit():
                ps_q = psA.tile([128, 512], F32, name="ps_q", tag="pa")
                e0 = et * 128
                for dj in range(ND):
                    nc.tensor.matmul(
                        ps_q,
                        lhsT=wqk_sb[:, dj, e0 : e0 + 128],
                        rhs=xt_sb[:, dj, :],
                        start=(dj == 0), stop=(dj == ND - 1),
                    )
                nc.vector.tensor_copy(qk_dest[et][:, blk], ps_q)
                if et == 2:
                    # D2 = partition-swapped copy of C2 (h2 self-pairing)
                    nc.sync.dma_start(out=D2[0:64, blk], in_=C2[64:128, blk])
                    nc.sync.dma_start(out=D2[64:128, blk], in_=C2[0:64, blk])
            return emit

        def v_chunk(tt):
            # V natural: stationary x^T tiles (full 128-wide), streaming Wv^T
            def emit():
                ps_v = psA.tile([128, HL * DK], F32, name="ps_v", tag="pa")
                tcol = tt * 128
                for dj in range(ND):
                    nc.tensor.matmul(
                        ps_v,
                        lhsT=xt_sb[:, dj, tcol : tcol + 128],
                        rhs=wv_sb[:, dj, :],
                        start=(dj == 0), stop=(dj == ND - 1),
                    )
                kt = tsb * 4 + tt
                nc.vector.tensor_copy(
                    V[:, kt, :, 0:DK],
                    ps_v.rearrange("p (h v) -> p h v", h=HL),
                )
            return emit

        return [qk_chunk(et) for et in range(3)] + [v_chunk(tt) for tt in range(4)]

    def emit_proj(tsb):
        for c in proj_chunks(tsb):
            c()

    def emit_normalize(h, psav, cols=slice(0, 512)):
        # out^T = psav / sums; sums sit in ones row 64.
        # (reciprocal_approx_fast is a custom DVE op and must read SBUF,
        # not PSUM — feeding it psav directly returns garbage.)
        pa = psav[h]
        w = cols.stop - cols.start
        sums_sb = spool.tile([1, 512], F32, name="sums_sb")
        nc.vector.tensor_copy(sums_sb[:, 0:w], pa[DK : DK + 1, cols])
        rt = spool.tile([1, 512], F32, name="rt")
        nc.vector.reciprocal_approx_fast(rt[:, 0:w], sums_sb[:, 0:w])
        rb = spool.tile([DK, 512], F32, name="rb")
        nc.gpsimd.partition_broadcast(rb[:, 0:w], rt[:, 0:w], channels=DK)
        if h == 0:
            nc.vector.tensor_mul(ot01[0:DK, cols], pa[0:DK, cols], rb[:, 0:w])
        elif h == 1:
            ot1s = spool.tile([DK, 512], BF16, name="ot1s")
            nc.vector.tensor_mul(ot1s[:, 0:w], pa[0:DK, cols], rb[:, 0:w])
            nc.sync.dma_start(out=ot01[DK:128, cols], in_=ot1s[:, 0:w])
        else:
            nc.vector.tensor_mul(ot2[:, cols], pa[0:DK, cols], rb[:, 0:w])

    def emit_attn_pass(qb, hpass, heads, psav, chunks=()):
        # chunks: deferred emission closures (projection pieces / out-proj)
        # interleaved one-per-kp so the PE queue stays fed while ScalarE
        # paces the exp stream.
        nk = 4 * (qb + 1)
        chunks = list(chunks)

        def emit_av(kts, pts):
            for h in heads:
                for i, kt in enumerate(kts):
                    # band tiles attend only to q >= 128*bp within the block
                    lo = (kt - 4 * qb) * 128 if kt >= 4 * qb else 0
                    off = slice(i * 512 + lo, (i + 1) * 512)
                    nc.tensor.matmul(
                        psav[h][:, lo:512],
                        lhsT=V[:, kt, h, :], rhs=pts[h][:, off],
                        start=(kt == 0), stop=(kt == nk - 1),
                    )

        qblk = qb * 512
        pend = None
        for kp in range(nk // 2):
            kt0, kt1 = 2 * kp, 2 * kp + 1
            ss = {h: psA.tile([128, 1024], F32, name=f"ss{h}", tag="pa")
                  for h in heads}
            for i, kt in enumerate((kt0, kt1)):
                kblk = slice(kt * 128, (kt + 1) * 128)
                lo = (kt - 4 * qb) * 128 if kt >= 4 * qb else 0
                off = slice(i * 512 + lo, (i + 1) * 512)
                qrng = slice(qblk + lo, qblk + 512)
                if hpass == 0:
                    nc.tensor.matmul(
                        ss[0][:, off], lhsT=KA[0:64, kblk],
                        rhs=QB[0:64, qrng], start=True, stop=True,
                    )
                    nc.tensor.matmul(
                        ss[1][:, off], lhsT=KA[64:128, kblk],
                        rhs=QB[64:128, qrng], start=True, stop=True,
                    )
                elif i == 0:
                    nc.tensor.matmul(
                        ss[2][:, off], lhsT=C2[0:64, kblk],
                        rhs=D2[0:64, qrng], start=True, stop=True,
                    )
                else:
                    nc.tensor.matmul(
                        ss[2][:, off], lhsT=D2[64:128, kblk],
                        rhs=C2[64:128, qrng], start=True, stop=True,
                    )
            pts = {}
            for h in heads:
                if hpass == 0 and h == 1 and kp % 2 == 1 and kp < 2 * qb:
                    # DVE bitcast-exp: int16 holding the bits of bf16 2^u.
                    # (exp of stale PSUM in the masked band strips is
                    # harmless: stale values are bounded scores/projections.)
                    pi = ptpool.tile([128, 1024], I16, name="pt")
                    nc.vector.tensor_scalar(
                        pi, ss[h], EXP_C1, EXP_C2, ALU.mult, ALU.add
                    )
                    pt = pi.bitcast(BF16)
                else:
                    pt = ptpool.tile([128, 1024], BF16, name="pt")
                    nc.scalar.activation(pt, ss[h], AF.Exp, scale=0.125)
                for i, kt in enumerate((kt0, kt1)):
                    if kt >= 4 * qb:  # diagonal band tile
                        off = slice(i * 512, (i + 1) * 512)
                        nc.vector.tensor_mul(
                            pt[:, off], pt[:, off], bandmask[kt - 4 * qb]
                        )
                pts[h] = pt
            if pend is not None:
                emit_av(*pend)
            pend = ((kt0, kt1), pts)
            if chunks and kp >= 2:
                chunks.pop(0)()
        emit_av(*pend)
        for c in chunks:
            c()

    def emit_outproj(qb, part, psy_tiles, ybig):
        # y^T[d, q]: heads 0/1 stacked on partition halves form one K=128
        # contraction (part 0); head 2's K=64 accumulates on top (part 1).
        # (Mixed ROW positions inside one accumulation group crash the HW,
        # so never pair row-groups within an accumulating chain.)
        qblk = slice(qb * 512, (qb + 1) * 512)
        for dj in range(ND):
            dblk = slice(dj * 128, (dj + 1) * 128)
            if part == 0:
                ps_y = psC.tile([128, 512], F32, name="ps_y", tag="pc")
                nc.tensor.matmul(
                    ps_y, lhsT=wo01_sb[:, dblk], rhs=ot01,
                    start=True, stop=False, skip_group_check=True,
                )
                psy_tiles[dj] = ps_y
            elif part == 1:  # head 2, first column half
                nc.tensor.matmul(
                    psy_tiles[dj][:, 0:256], lhsT=wo2_sb[:, dblk],
                    rhs=ot2[:, 0:256],
                    start=False, stop=False, skip_group_check=True,
                )
            else:  # head 2, second half + drain
                ps_y = psy_tiles[dj]
                nc.tensor.matmul(
                    ps_y[:, 256:512], lhsT=wo2_sb[:, dblk],
                    rhs=ot2[:, 256:512],
                    start=False, stop=True, skip_group_check=True,
                )
                nc.vector.tensor_copy(ybig[:, dj, :], ps_y)
        if part == 2:
            nc.sync.dma_start(out=y_v[:, :, qblk], in_=ybig)

    emit_proj(0)
    for qb in range(NQB):
        psav = {}
        psav[0] = psB.tile([DK + 1, 512], F32, name="psav0", tag="pb")
        psav[1] = psB.tile([DK + 1, 512], F32, name="psav1", tag="pb")
        ch = proj_chunks(qb + 1) if qb + 1 < NQB else []
        emit_attn_pass(qb, 0, (0, 1), psav)
        emit_normalize(0, psav)
        emit_normalize(1, psav)
        psav[2] = psB.tile([DK + 1, 512], F32, name="psav2", tag="pb")
        psy_tiles = {}
        ybig = ypool.tile([128, ND, 512], BF16, name="ybig")
        # outproj part 0 (needs only ot01) rides as the trailing chunk of
        # hpass1 so the PE has work while the h2 normalize chain runs
        emit_attn_pass(
            qb, 1, (2,), psav,
            ch + [lambda: emit_outproj(qb, 0, psy_tiles, ybig)],
        )
        emit_normalize(2, psav, slice(0, 256))
        emit_outproj(qb, 1, psy_tiles, ybig)
        emit_normalize(2, psav, slice(256, 512))
        emit_outproj(qb, 2, psy_tiles, ybig)
    ctx.close()


def build():
    if "nc" in _CACHE:
        return _CACHE["nc"]
    nc = bacc.Bacc(
        "TRN2", target_bir_lowering=False, debug=False, num_devices=NCORES
    )
    with tile.TileContext(nc) as tc:
        _emit(tc)
    nc.compile()
    _CACHE["nc"] = nc
    return nc


def make_in_maps(x, w_qkv, w_out):
    x = np.asarray(x, dtype=np.float32)
    w_qkv = np.asarray(w_qkv, dtype=np.float32)
    w_out = np.asarray(w_out, dtype=np.float32)
    wq = w_qkv[0:D]        # [768, 768], rows = q features
    wk = w_qkv[D : 2 * D]
    wv = w_qkv[2 * D :]
    xT = [np.ascontiguousarray(x[b].T).astype(BF) for b in range(B)]
    in_maps = []
    for c in range(NCORES):
        b, g = divmod(c, 4)
        hs = [3 * g + j for j in range(HL)]  # global head ids
        h0, h1, h2 = hs
        cols = []
        for pair in ((wk, h0), (wk, h1), (wq, h0), (wq, h1), (wk, h2), (wq, h2)):
            w, h = pair
            cols.append(w[h * DK : (h + 1) * DK].T)  # [768, 64]
        wqkT = np.ascontiguousarray(np.concatenate(cols, axis=1))  # [768, 384]
        wvT = np.ascontiguousarray(
            np.concatenate([wv[h * DK : (h + 1) * DK].T for h in hs], axis=1)
        )  # [768, 192]
        woT = np.ascontiguousarray(
            np.stack([w_out[:, h * DK : (h + 1) * DK].T for h in hs])
        )  # [3, 64, 768]
        in_maps.append(
            {
                "xT": xT[b],
                "wqkT": wqkT.astype(BF),
                "wvT": wvT.astype(BF),
                "woT": woT.astype(BF),
            }
        )
    return in_maps


def run(inputs, trace=False):
    """Run on hardware; returns (y [B,T,D] fp32, BassKernelResults)."""
    nc = build()
    in_maps = make_in_maps(inputs["x"], inputs["w_qkv"], inputs["w_out"])
    br = run_bass_kernel_spmd(nc, in_maps, list(range(NCORES)), trace=trace)
    y = np.zeros((B, T, D), dtype=np.float32)
    for c in range(NCORES):
        b = c // 4
        y[b] += np.asarray(br.results[c]["yT"]).astype(np.float32).T
    return y, br


def kernel(x, w_qkv, w_out):
    y, _ = run({"x": x, "w_qkv": w_qkv, "w_out": w_out})
    return y


# revision 60
# speedup vs baseline: 1.0133x; 1.0133x over previous
"""Multi-head causal self-attention (B=2, T=4096, D=768, H=12) on 8 trn2 cores.

Sharding: core c -> batch b = c//4, heads 3*(c%4) .. 3*(c%4)+2.
qkv_proj column-parallel (each core computes Q/K/V only for its heads),
out_proj row-parallel (each core emits a partial y^T; host sums the 4
partials per batch).

Device dataflow (bf16 operands, fp32 PSUM accumulation):
  x^T pre-transposed and cast to bf16 on the host -> Q^T/K^T via
  transposed projection (W^T stationary 128-wide, x^T streaming) ->
  S^T = K Q^T in [k,q] layout, heads 0/1 row-paired on opposite PE
  halves, head 2 self-paired via a partition-swapped copy; diagonal
  band tiles stream only their causally live q columns -> exp straight
  out of PSUM into bf16 SBUF on ScalarE, except head 1 on odd non-band
  k-pairs which uses a DVE integer-bitcast 2^u approx (one tensor_scalar
  into int16 whose bits are the bf16 exp; the +-3% sawtooth cancels in
  the softmax normalization) so neither engine paces the pipeline ->
  causal band masks on DVE (all-SBUF bf16) -> out^T = V^T P^T with a
  ones column appended to V collecting softmax denominators in
  accumulator row 64 -> normalize via approx reciprocal + gpsimd
  partition broadcast -> y^T partial = Wo^T.T out^T with heads 0/1
  row-paired, stored bf16.

Emission order per step: attention hpass0 (h0/h1) -> normalize h0/h1
-> hpass1 (h2, with the next superblock's projection work and this
q-block's first out-proj half interleaved one chunk per k-pair) ->
normalize h2 -> out-proj. The PE queue therefore always has projection
work while the exp/normalize chains run, and AV matmuls lag one k-pair
behind scores so the PE never head-of-line blocks on the exp."""

import sys

sys.path.insert(0, "/opt/trn_rl_repo")

import numpy as np
import ml_dtypes
from contextlib import ExitStack

import concourse.bass as bass
import concourse.bacc as bacc
import concourse.tile as tile
import concourse.mybir as mybir
from concourse.bass_utils import run_bass_kernel_spmd

F32 = mybir.dt.float32
BF16 = mybir.dt.bfloat16
I16 = mybir.dt.int16
AF = mybir.ActivationFunctionType
ALU = mybir.AluOpType
BF = ml_dtypes.bfloat16

B = 2
T = 4096
D = 768
H = 12
DK = 64
NCORES = 8
HL = 3  # heads per core
ND = D // 128  # 6 d-tiles
NKT = T // 128  # 32 k-tiles
NQB = T // 512  # 8 q-blocks
NTSB = T // 512  # 8 t-superblocks (4 t-tiles each)

# DVE bitcast-exp constants: bf16bits(exp(s)) ~ round(s*0.125*log2e*128
# + (16256 - 5.5)); the -5.5 centers the linear-mantissa sawtooth.
EXP_C1 = 0.125 * 1.4426950408889634 * 128.0
EXP_C2 = 16256.0 - 5.5

_CACHE = {}


def _emit(tc):
    nc = tc.nc
    xt_d = nc.dram_tensor("xT", [D, T], BF16, kind="ExternalInput").ap()
    wqk_d = nc.dram_tensor("wqkT", [D, 384], BF16, kind="ExternalInput").ap()
    wv_d = nc.dram_tensor("wvT", [D, HL * DK], BF16, kind="ExternalInput").ap()
    wo_d = nc.dram_tensor("woT", [HL, DK, D], BF16, kind="ExternalInput").ap()
    y_d = nc.dram_tensor("yT", [D, T], BF16, kind="ExternalOutput").ap()
    xt_v = xt_d.rearrange("(j p) t -> p j t", p=128)
    y_v = y_d.rearrange("(j p) t -> p j t", p=128)

    ctx = ExitStack()
    const = ctx.enter_context(tc.tile_pool(name="const", bufs=1))
    persist = ctx.enter_context(tc.tile_pool(name="persist", bufs=1))
    xtpool = ctx.enter_context(tc.tile_pool(name="xt", bufs=2))
    ptpool = ctx.enter_context(tc.tile_pool(name="pt", bufs=6))
    spool = ctx.enter_context(tc.tile_pool(name="sp", bufs=3))
    ypool = ctx.enter_context(tc.tile_pool(name="yp", bufs=2))
    # PSUM (8 banks): pa = streaming (S tiles, qkv, V) 2 bufs x 2 banks;
    # pb = AV accumulators, 2 bufs (h2 reuses h0's slot after normalize
    # h0); pc = yT out-proj, 2 bufs so dj pipelines across the copy.
    psA = ctx.enter_context(tc.tile_pool(name="psA", bufs=2, space="PSUM"))
    psB = ctx.enter_context(tc.tile_pool(name="psB", bufs=2, space="PSUM"))
    psC = ctx.enter_context(tc.tile_pool(name="psC", bufs=2, space="PSUM"))

    # ---- constants ----
    # causal band masks for the 4 diagonal-band k-tiles of each q-block:
    # bandmask[bp][k, q] = 0 for q < 128*bp + k, else 1
    bandmask = []
    for bp in range(4):
        m = const.tile([128, 512], BF16, name=f"bandmask{bp}")
        nc.gpsimd.memset(m, 1.0)
        nc.gpsimd.affine_select(
            out=m, in_=m, compare_op=mybir.AluOpType.is_ge, fill=0.0,
            base=-128 * bp, pattern=[[1, 512]], channel_multiplier=-1,
        )
        bandmask.append(m)

    wqk_sb = const.tile([128, ND, 384], BF16)
    nc.sync.dma_start(out=wqk_sb, in_=wqk_d.rearrange("(j p) e -> p j e", p=128))
    wv_sb = const.tile([128, ND, HL * DK], BF16)
    nc.sync.dma_start(out=wv_sb, in_=wv_d.rearrange("(j p) e -> p j e", p=128))
    wo01_sb = const.tile([128, D], BF16)  # head0 rows on 0:64, head1 on 64:128
    nc.sync.dma_start(out=wo01_sb, in_=wo_d[0:2].rearrange("h p d -> (h p) d"))
    wo2_sb = const.tile([DK, D], BF16)
    nc.sync.dma_start(out=wo2_sb, in_=wo_d[2])

    # ---- persistent activations ----
    # KA: [K^T_h0 ; K^T_h1], QB: [Q^T_h0 ; Q^T_h1] on partition halves
    KA = persist.tile([128, T], BF16, name="KA")
    QB = persist.tile([128, T], BF16, name="QB")
    C2 = persist.tile([128, T], BF16, name="C2")  # [K^T_h2 ; Q^T_h2]
    D2 = persist.tile([128, T], BF16, name="D2")  # [Q^T_h2 ; K^T_h2] (swapped)
    # V natural layout [k-part, kt, head, v(64)+ones]; the ones column
    # collects softmax denominators in AV accumulator row 64
    V = persist.tile([128, NKT, HL, DK + 1], BF16, name="V")
    for h in range(HL):
        nc.gpsimd.memset(V[:, :, h, DK : DK + 1], 1.0)
    ot01 = persist.tile([128, 512], BF16, name="ot01")  # heads 0/1 out^T per qb
    ot2 = persist.tile([DK, 512], BF16, name="ot2")

    qk_dest = [KA, QB, C2]

    def proj_chunks(tsb):
        """Emit the x^T DMA now; return per-(e|t)-tile projection closures
        for interleaved emission."""
        blk = slice(tsb * 512, (tsb + 1) * 512)
        xt_sb = xtpool.tile([128, ND, 512], BF16, name="xt_sb")
        nc.sync.dma_start(out=xt_sb, in_=xt_v[:, :, blk])

        def qk_chunk(et):
            # Q^T / K^T projection: out[e, t] block per e-tile (full 128-wide
            # stationary: e-tile 0 = [K_h0|K_h1], 1 = [Q_h0|Q_h1], 2 = [K_h2|Q_h2])
            def emit():
                ps_q = psA.tile([128, 512], F32, name="ps_q", tag="pa")
                e0 = et * 128
                for dj in range(ND):
                    nc.tensor.matmul(
                        ps_q,
                        lhsT=wqk_sb[:, dj, e0 : e0 + 128],
                        rhs=xt_sb[:, dj, :],
                        start=(dj == 0), stop=(dj == ND - 1),
                    )
                nc.vector.tensor_copy(qk_dest[et][:, blk], ps_q)
                if et == 2:
                    # D2 = partition-swapped copy of C2 (h2 self-pairing)
                    nc.sync.dma_start(out=D2[0:64, blk], in_=C2[64:128, blk])
                    nc.sync.dma_start(out=D2[64:128, blk], in_=C2[0:64, blk])
            return emit

        def v_chunk(tt):
            # V natural: stationary x^T tiles (full 128-wide), streaming Wv^T
            def emit():
                ps_v = psA.tile([128, HL * DK], F32, name="ps_v", tag="pa")
                tcol = tt * 128
                for dj in range(ND):
                    nc.tensor.matmul(
                        ps_v,
                        lhsT=xt_sb[:, dj, tcol : tcol + 128],
                        rhs=wv_sb[:, dj, :],
                        start=(dj == 0), stop=(dj == ND - 1),
                    )
                kt = tsb * 4 + tt
                nc.vector.tensor_copy(
                    V[:, kt, :, 0:DK],
                    ps_v.rearrange("p (h v) -> p h v", h=HL),
                )
            return emit

        return [qk_chunk(et) for et in range(3)] + [v_chunk(tt) for tt in range(4)]

    def emit_proj(tsb):
        for c in proj_chunks(tsb):
            c()

    def emit_normalize(h, psav, cols=slice(0, 512)):
        # out^T = psav / sums; sums sit in ones row 64.
        # (reciprocal_approx_fast is a custom DVE op and must read SBUF,
        # not PSUM — feeding it psav directly returns garbage.)
        pa = psav[h]
        w = cols.stop - cols.start
        sums_sb = spool.tile([1, 512], F32, name="sums_sb")
        nc.vector.tensor_copy(sums_sb[:, 0:w], pa[DK : DK + 1, cols])
        rt = spool.tile([1, 512], F32, name="rt")
        nc.vector.reciprocal_approx_fast(rt[:, 0:w], sums_sb[:, 0:w])
        rb = spool.tile([DK, 512], F32, name="rb")
        nc.gpsimd.partition_broadcast(rb[:, 0:w], rt[:, 0:w], channels=DK)
        if h == 0:
            nc.vector.tensor_mul(ot01[0:DK, cols], pa[0:DK, cols], rb[:, 0:w])
        elif h == 1:
            ot1s = spool.tile([DK, 512], BF16, name="ot1s")
            nc.vector.tensor_mul(ot1s[:, 0:w], pa[0:DK, cols], rb[:, 0:w])
            nc.sync.dma_start(out=ot01[DK:128, cols], in_=ot1s[:, 0:w])
        else:
            nc.vector.tensor_mul(ot2[:, cols], pa[0:DK, cols], rb[:, 0:w])

    def emit_attn_pass(qb, hpass, heads, psav, chunks=()):
        # chunks: deferred emission closures (projection pieces / out-proj)
        # interleaved one-per-kp so the PE queue stays fed while ScalarE
        # paces the exp stream.
        nk = 4 * (qb + 1)
        chunks = list(chunks)

        def emit_av(kts, pts):
            for h in heads:
                for i, kt in enumerate(kts):
                    # band tiles attend only to q >= 128*bp within the block
                    lo = (kt - 4 * qb) * 128 if kt >= 4 * qb else 0
                    off = slice(i * 512 + lo, (i + 1) * 512)
                    nc.tensor.matmul(
                        psav[h][:, lo:512],
                        lhsT=V[:, kt, h, :], rhs=pts[h][:, off],
                        start=(kt == 0), stop=(kt == nk - 1),
                    )

        qblk = qb * 512
        pend = None
        for kp in range(nk // 2):
            kt0, kt1 = 2 * kp, 2 * kp + 1
            ss = {h: psA.tile([128, 1024], F32, name=f"ss{h}", tag="pa")
                  for h in heads}
            for i, kt in enumerate((kt0, kt1)):
                kblk = slice(kt * 128, (kt + 1) * 128)
                lo = (kt - 4 * qb) * 128 if kt >= 4 * qb else 0
                off = slice(i * 512 + lo, (i + 1) * 512)
                qrng = slice(qblk + lo, qblk + 512)
                if hpass == 0:
                    nc.tensor.matmul(
                        ss[0][:, off], lhsT=KA[0:64, kblk],
                        rhs=QB[0:64, qrng], start=True, stop=True,
                    )
                    nc.tensor.matmul(
                        ss[1][:, off], lhsT=KA[64:128, kblk],
                        rhs=QB[64:128, qrng], start=True, stop=True,
                    )
                elif i == 0:
                    nc.tensor.matmul(
                        ss[2][:, off], lhsT=C2[0:64, kblk],
                        rhs=D2[0:64, qrng], start=True, stop=True,
                    )
                else:
                    nc.tensor.matmul(
                        ss[2][:, off], lhsT=D2[64:128, kblk],
                        rhs=C2[64:128, qrng], start=True, stop=True,
                    )
            pts = {}
            for h in heads:
                if hpass == 0 and h == 1 and kp % 2 == 1 and kp < 2 * qb:
                    # DVE bitcast-exp: int16 holding the bits of bf16 2^u.
                    # (exp of stale PSUM in the masked band strips is
                    # harmless: stale values are bounded scores/projections.)
                    pi = ptpool.tile([128, 1024], I16, name="pt")
                    nc.vector.tensor_scalar(
                        pi, ss[h], EXP_C1, EXP_C2, ALU.mult, ALU.add
                    )
                    pt = pi.bitcast(BF16)
                else:
                    pt = ptpool.tile([128, 1024], BF16, name="pt")
                    nc.scalar.activation(pt, ss[h], AF.Exp, scale=0.125)
                for i, kt in enumerate((kt0, kt1)):
                    if kt >= 4 * qb:  # diagonal band tile
                        off = slice(i * 512, (i + 1) * 512)
                        nc.vector.tensor_mul(
                            pt[:, off], pt[:, off], bandmask[kt - 4 * qb]
                        )
                pts[h] = pt
            if pend is not None:
                emit_av(*pend)
            pend = ((kt0, kt1), pts)
            if chunks and kp >= 2:
                chunks.pop(0)()
        emit_av(*pend)
        for c in chunks:
            c()

    def emit_outproj(qb, part, psy_tiles, ybig):
        # y^T[d, q]: heads 0/1 stacked on partition halves form one K=128
        # contraction (part 0); head 2's K=64 accumulates on top (part 1).
        # (Mixed ROW positions inside one accumulation group crash the HW,
        # so never pair row-groups within an accumulating chain.)
        qblk = slice(qb * 512, (qb + 1) * 512)
        for dj in range(ND):
            dblk = slice(dj * 128, (dj + 1) * 128)
            if part == 0:
                ps_y = psC.tile([128, 512], F32, name="ps_y", tag="pc")
                nc.tensor.matmul(
                    ps_y, lhsT=wo01_sb[:, dblk], rhs=ot01,
                    start=True, stop=False, skip_group_check=True,
                )
                psy_tiles[dj] = ps_y
            elif part == 1:  # head 2, first column half
                nc.tensor.matmul(
                    psy_tiles[dj][:, 0:256], lhsT=wo2_sb[:, dblk],
                    rhs=ot2[:, 0:256],
                    start=False, stop=False, skip_group_check=True,
                )
            else:  # head 2, second half + drain
                ps_y = psy_tiles[dj]
                nc.tensor.matmul(
                    ps_y[:, 256:512], lhsT=wo2_sb[:, dblk],
                    rhs=ot2[:, 256:512],
                    start=False, stop=True, skip_group_check=True,
                )
                nc.vector.tensor_copy(ybig[:, dj, :], ps_y)
        if part == 2:
            nc.sync.dma_start(out=y_v[:, :, qblk], in_=ybig)

    emit_proj(0)
    for qb in range(NQB):
        psav = {}
        psav[0] = psB.tile([DK + 1, 512], F32, name="psav0", tag="pb")
        psav[1] = psB.tile([DK + 1, 512], F32, name="psav1", tag="pb")
        ch = proj_chunks(qb + 1) if qb + 1 < NQB else []
        emit_attn_pass(qb, 0, (0, 1), psav)
        emit_normalize(0, psav)
        emit_normalize(1, psav)
        psav[2] = psB.tile([DK + 1, 512], F32, name="psav2", tag="pb")
        psy_tiles = {}
        ybig = ypool.tile([128, ND, 512], BF16, name="ybig")
        # outproj part 0 (needs only ot01) rides as the trailing chunk of
        # hpass1 so the PE has work while the h2 normalize chain runs
        emit_attn_pass(
            qb, 1, (2,), psav,
            ch + [lambda: emit_outproj(qb, 0, psy_tiles, ybig)],
        )
        emit_normalize(2, psav, slice(0, 256))
        emit_outproj(qb, 1, psy_tiles, ybig)
        emit_normalize(2, psav, slice(256, 512))
        emit_outproj(qb, 2, psy_tiles, ybig)
    ctx.close()


def build():
    if "nc" in _CACHE:
        return _CACHE["nc"]
    nc = bacc.Bacc(
        "TRN2", target_bir_lowering=False, debug=False, num_devices=NCORES
    )
    with tile.TileContext(nc) as tc:
        _emit(tc)
    nc.compile()
    _CACHE["nc"] = nc
    return nc


def make_in_maps(x, w_qkv, w_out):
    x = np.asarray(x, dtype=np.float32)
    w_qkv = np.asarray(w_qkv, dtype=np.float32)
    w_out = np.asarray(w_out, dtype=np.float32)
    wq = w_qkv[0:D]        # [768, 768], rows = q features
    wk = w_qkv[D : 2 * D]
    wv = w_qkv[2 * D :]
    xT = [np.ascontiguousarray(x[b].T).astype(BF) for b in range(B)]
    in_maps = []
    for c in range(NCORES):
        b, g = divmod(c, 4)
        hs = [3 * g + j for j in range(HL)]  # global head ids
        h0, h1, h2 = hs
        cols = []
        for pair in ((wk, h0), (wk, h1), (wq, h0), (wq, h1), (wk, h2), (wq, h2)):
            w, h = pair
            cols.append(w[h * DK : (h + 1) * DK].T)  # [768, 64]
        wqkT = np.ascontiguousarray(np.concatenate(cols, axis=1))  # [768, 384]
        wvT = np.ascontiguousarray(
            np.concatenate([wv[h * DK : (h + 1) * DK].T for h in hs], axis=1)
        )  # [768, 192]
        woT = np.ascontiguousarray(
            np.stack([w_out[:, h * DK : (h + 1) * DK].T for h in hs])
        )  # [3, 64, 768]
        in_maps.append(
            {
                "xT": xT[b],
                "wqkT": wqkT.astype(BF),
                "wvT": wvT.astype(BF),
                "woT": woT.astype(BF),
            }
        )
    return in_maps


def run(inputs, trace=False):
    """Run on hardware; returns (y [B,T,D] fp32, BassKernelResults)."""
    nc = build()
    in_maps = make_in_maps(inputs["x"], inputs["w_qkv"], inputs["w_out"])
    br = run_bass_kernel_spmd(nc, in_maps, list(range(NCORES)), trace=trace)
    y = np.zeros((B, T, D), dtype=np.float32)
    for c in range(NCORES):
        b = c // 4
        y[b] += np.asarray(br.results[c]["yT"]).astype(np.float32).T
    return y, br


def kernel(x, w_qkv, w_out):
    y, _ = run({"x": x, "w_qkv": w_qkv, "w_out": w_out})
    return y
